# revision 26
# baseline (speedup 1.0000x reference)
"""Trainium2 Bass kernel for the DelayedXOR-SH-SNN problem.

Reference semantics (per batch b, hidden h, fp32 throughout):
    ic[t] = x[b,t,:] @ W1[h,:] + b1[h]
    v_t   = alpha_h * v_{t-1} + (1-alpha_h) * ic[t] - s_{t-1}        (V_TH = 1)
    s_t   = (v_t - 1 > 0)
    out[b] = sum_{t >= T/2} s_t @ W2.T + b2

Strategy: pure data-parallel over batch (8 cores x 128 batches).  Per core:
  - x is pre-arranged on the host into xt8[128, 32768]:
        row = (t % 8)*16 + i,  col = (t // 8)*128 + b
    One TensorE matmul with a block-diagonal lhsT (K=128 = 8 interleaved
    timesteps x 16 inputs, M=128 = 2 timesteps x 64 h) produces
    c'(t) = (1-alpha)*(x@W1) for two timesteps of all 128 batches in PSUM,
    layout [(2t, 64 h), 128 b].
  - The recurrence runs serially over t on the Vector engine with state
    v,s [64,128]; alpha enters as a per-partition scalar.
  - Spike counts accumulate for t >= T/2; final out = W2 @ acc + b2 via one
    TensorE matmul reduced over the 64 h partitions.

The walrus build in this container encodes at most ONE sync-wait command per
TPB instruction; Tile attaches several.  _split_multi_waits() legalizes the
program post-scheduling by hoisting all but one wait of each instruction into
standalone NoOps on the same engine queue.
"""

from contextlib import ExitStack

import numpy as np

import concourse.bass as bass
import concourse.mybir as mybir
from concourse.tile import TileContext

N_CORES = 8
B, T, I, H = 1024, 2048, 16, 64
BL = B // N_CORES  # batches per core
NJ = 8             # timestep interleave in the x layout


def _split_multi_waits(nc, max_waits=1):
    """Hoist surplus sync waits into standalone NoOps (1 wait slot per TPB
    instruction in this walrus build)."""
    for func in nc.m.functions:
        for block in func.blocks:
            insts = list(block.instructions)
            out = []
            changed = False
            for inst in insts:
                si = getattr(inst, "sync_info", None)
                waits = list(si.on_wait) if si is not None and si.on_wait else []
                if len(waits) > max_waits:
                    keep = waits[-max_waits:]
                    hoist = waits[:-max_waits]
                    for k, w in enumerate(hoist):
                        nop = mybir.InstNoOp(
                            name=f"{inst.name}-wait{k}", engine=inst.engine
                        )
                        nop.sync_info = mybir.SyncInfo(on_wait=[w], on_update=[])
                        out.append(nop)
                    si.on_wait = keep
                    changed = True
                out.append(inst)
            if changed:
                block.instructions = out
    return nc


def _build_program(t_steps=T, add_b1=False):
    tgrp = t_steps // NJ
    cols = BL * tgrp
    f32 = mybir.dt.float32
    nc = bass.Bass()

    xt = nc.declare_dram_parameter("xt", [NJ * I, cols], f32, isOutput=False)
    w1p = nc.declare_dram_parameter("w1p", [NJ * I, 4 * NJ * I], f32, isOutput=False)
    alpha = nc.declare_dram_parameter("alpha", [H, 1], f32, isOutput=False)
    b1p = nc.declare_dram_parameter("b1p", [1, NJ * I], f32, isOutput=False)
    w2 = nc.declare_dram_parameter("w2", [H, 1], f32, isOutput=False)
    b2 = nc.declare_dram_parameter("b2", [1, 1], f32, isOutput=False)
    out = nc.declare_dram_parameter("out", [1, BL], f32, isOutput=True)

    with TileContext(nc) as tc, ExitStack() as ctx:
        xpool = ctx.enter_context(tc.tile_pool(name="x", bufs=1))
        cpool = ctx.enter_context(tc.tile_pool(name="consts", bufs=1))
        spool = ctx.enter_context(tc.tile_pool(name="state", bufs=1))
        ppool = ctx.enter_context(tc.tile_pool(name="psum", bufs=7, space="PSUM"))
        opool = ctx.enter_context(tc.tile_pool(name="opsum", bufs=1, space="PSUM"))
        upool = ctx.enter_context(tc.tile_pool(name="u", bufs=3))

        xt_t = xpool.tile([NJ * I, cols], f32)
        ncol_dma = cols // NJ
        for j in range(NJ):
            nc.sync.dma_start(
                xt_t[:, ncol_dma * j : ncol_dma * (j + 1)],
                xt[:, ncol_dma * j : ncol_dma * (j + 1)],
            )

        w1p_t = cpool.tile([NJ * I, 4 * NJ * I], f32)
        nc.sync.dma_start(w1p_t[:], w1p[:])
        alpha_t = cpool.tile([H, 1], f32)
        nc.sync.dma_start(alpha_t[:], alpha[:])
        b1p_t = cpool.tile([1, NJ * I], f32)
        nc.sync.dma_start(b1p_t[:], b1p[:])
        w2_t = cpool.tile([H, 1], f32)
        nc.sync.dma_start(w2_t[:], w2[:])
        b2_t = cpool.tile([1, 1], f32)
        nc.sync.dma_start(b2_t[:], b2[:])
        ones_t = cpool.tile([1, BL], f32)
        nc.vector.memset(ones_t[:], 1.0)

        v_t = spool.tile([H, BL], f32, tag="v")
        s_t = spool.tile([H, BL], f32, tag="s")
        acc_t = spool.tile([H, BL], f32, tag="acc")
        nc.vector.memset(v_t[:], 0.0)
        nc.vector.memset(s_t[:], 0.0)
        nc.vector.memset(acc_t[:], 0.0)

        for tp in range(t_steps // 2):
            # one matmul computes c' for timesteps (2*tp, 2*tp+1):
            # PSUM [(t'=2) x (h=64), b=128]
            tg, k = divmod(tp, 4)
            cp = ppool.tile([2 * H, BL], f32, tag="cp")
            nc.tensor.matmul(
                cp[:], lhsT=w1p_t[:, 2 * H * k : 2 * H * (k + 1)],
                rhs=xt_t[:, BL * tg : BL * (tg + 1)],
                start=True, stop=not add_b1,
            )
            if add_b1:
                nc.tensor.matmul(
                    cp[:], lhsT=b1p_t[:], rhs=ones_t[:],
                    start=False, stop=True,
                )
            for tsub in range(2):
                t = 2 * tp + tsub
                cslice = cp[H * tsub : H * (tsub + 1), :]
                # u = c' - s_{t-1}
                u_t = upool.tile([H, BL], f32, tag="u")
                nc.vector.tensor_tensor(
                    out=u_t[:], in0=cslice, in1=s_t[:],
                    op=mybir.AluOpType.subtract,
                )
                # v = alpha*v + u
                nc.vector.scalar_tensor_tensor(
                    out=v_t[:], in0=v_t[:], scalar=alpha_t[:], in1=u_t[:],
                    op0=mybir.AluOpType.mult, op1=mybir.AluOpType.add,
                )
                # s_t = (v > 1)
                nc.vector.tensor_scalar(
                    out=s_t[:], in0=v_t[:], scalar1=1.0, scalar2=None,
                    op0=mybir.AluOpType.is_gt,
                )
                if t >= t_steps // 2:
                    nc.vector.tensor_add(out=acc_t[:], in0=acc_t[:], in1=s_t[:])

        op = opool.tile([1, BL], f32, tag="out")
        nc.tensor.matmul(op[:], lhsT=w2_t[:], rhs=acc_t[:], start=True, stop=True)
        ob = cpool.tile([1, BL], f32)
        nc.scalar.activation(
            out=ob[:], in_=op[:], func=mybir.ActivationFunctionType.Identity,
            bias=b2_t[:, 0:1], scale=1.0,
        )
        nc.sync.dma_start(out[:], ob[:])

    return _split_multi_waits(nc)


def _host_prep(x, W1, b1, tau_m, W2, b2, t_steps=T):
    tgrp = t_steps // NJ  # number of 8-timestep groups
    alpha = (1.0 / (1.0 + np.exp(-tau_m.astype(np.float64)))).astype(np.float32)
    one_m_a = (1.0 - alpha).astype(np.float32)
    w1s = (one_m_a[:, None] * W1).T.astype(np.float32)  # [I, H]
    b1s = (one_m_a * b1).astype(np.float32)

    # block-diagonal lhsT: w1p[tm*16+i, k*128 + tsub*64 + h] = w1s[i,h]
    # iff tm == 2k + tsub
    w1p = np.zeros((NJ * I, 4 * NJ * I), np.float32)
    for k in range(4):
        for tsub in range(2):
            tm = 2 * k + tsub
            w1p[tm * I : (tm + 1) * I, k * 128 + tsub * H : k * 128 + (tsub + 1) * H] = w1s
    b1p = np.tile(b1s, 2).reshape(1, 2 * H).astype(np.float32)

    w2c = np.ascontiguousarray(W2.reshape(1, H).T.astype(np.float32))  # [H, 1]
    b2c = np.asarray(b2, np.float32).reshape(1, 1)
    alc = alpha.reshape(H, 1)

    in_maps = []
    for c in range(N_CORES):
        xs = x[c * BL : (c + 1) * BL, :t_steps, :]                # [BL, T, I]
        arr = xs.transpose(1, 2, 0)                                # [T, I, BL]
        arr = arr.reshape(tgrp, NJ, I, BL).transpose(1, 2, 0, 3)   # (tm, i, tg, b)
        xt8 = np.ascontiguousarray(arr.reshape(NJ * I, tgrp * BL), np.float32)
        in_maps.append(
            {"xt": xt8, "w1p": w1p, "alpha": alc, "b1p": b1p, "w2": w2c, "b2": b2c}
        )
    return in_maps


_PROGRAM_CACHE = {}


def kernel(x, W1, b1, tau_m, W2, b2, _trace=False):
    x = np.asarray(x, np.float32)
    W1 = np.asarray(W1, np.float32)
    b1 = np.asarray(b1, np.float32)
    tau_m = np.asarray(tau_m, np.float32)
    W2 = np.asarray(W2, np.float32)
    b2 = np.asarray(b2, np.float32)

    from concourse.bass_utils import run_bass_kernel_spmd

    add_b1 = bool(np.any(b1 != 0.0))
    key = (T, add_b1)
    if key not in _PROGRAM_CACHE:
        _PROGRAM_CACHE[key] = _build_program(T, add_b1=add_b1)
    nc = _PROGRAM_CACHE[key]

    in_maps = _host_prep(x, W1, b1, tau_m, W2, b2)
    res = run_bass_kernel_spmd(nc, in_maps, list(range(N_CORES)), trace=_trace)
    outs = [np.asarray(res.results[c]["out"]).reshape(BL) for c in range(N_CORES)]
    full = np.concatenate(outs).astype(np.float32).reshape(B, 1)
    if _trace:
        kernel._last_results = res
    return full
